# revision 26
# baseline (speedup 1.0000x reference)
"""Causal cosine-sim attention (qk rmsnorm, scale=8) on 8 trn2 NeuronCores.

Shapes: q,k,v [2,16,2048,64] fp32; out [2,16,2048,64] fp32.
Sharding: 32 (batch, head) pairs -> 4 per core (head-parallel); each core
runs an identical SPMD program on its own 4 heads.

v2 changes over the 143us baseline (trace-driven):
  - startup: first matmul was at 43us (norm chain + scratch round trip
    fully serial).  h0 now runs a split k-chain -> q-chain so kT's
    scratch/transpose DMAs overlap q's DVE work; PE warm-up matmuls keep
    the HAM clock at 2.4GHz from ~12us instead of 60us.
  - scratch write: single DMA with a dup-read source AP (stride-0 dup
    axis) -> DRAM-side 4KB contiguous bursts instead of 2 DMAs of
    scattered 128B lines (measured 98GB/s -> line rate).
  - S^T and PV matmuls issue as single <=1024-col bf16 moving operands
    spanning two PSUM banks (halves MM count; per-MM overhead ~60ns).
  - output: divide writes bf16, one DMA per half in direct [p,t,d]
    layout (contiguous 1KB/partition); host reassembles + casts.
"""

import sys
from collections import deque

import numpy as np

try:
    import concourse.bass as bass
except ImportError:
    sys.path.insert(0, "/opt/trn_rl_repo")
    import concourse.bass as bass

import concourse.mybir as mybir
import concourse.tile as tile
from concourse import bacc
from concourse.bass_utils import run_bass_kernel_spmd
from concourse.masks import make_identity

FP32 = mybir.dt.float32
BF16 = mybir.dt.bfloat16

N_CORES = 8
B, H, S, D = 2, 16, 2048, 64
HPC = (B * H) // N_CORES  # heads per core = 4
P = 128
NT = S // P  # 16 key/query blocks
HALF = S // 2
COSINE_SIM_SCALE = 8.0

SPAN2 = False  # 1024-col moving matmuls rejected by ISA (s3d3_mm_num_elements)
# PE warm-up matmuls: sized so junk ENDS as preprocessing finishes; junk
# overshoot queues ahead of real matmuls (FIFO) and costs far more than
# the ~2us cold-start it saves.  0 disables.
N_WARM = 0
# PE turned out to be the steady-state pacer (~160% overlapped busy, 7us
# of gaps in a 65us window) -- adding 16 mask matmuls/head to it was a
# loss; the DVE tri-mul runs on a 64%-idle engine instead.
DIAG_PE = False
OFF_TAILS = True  # tail-unit exp on DVE (Schraudolph int16 bitcast)
MASKVAL = -8.0  # pre-exp additive mask: exp(4*(x-8)) <= e^-16 ~ 0
LOG2E = 1.4426950408889634
SCH_A = 128.0 * 4.0 * LOG2E  # Schraudolph scale (includes the exp arg x4)
SCH_B = 128.0 * (127.0 - 0.0579)  # bias, sigma balances max rel err


def build_nc(use_scale=True, use_mask=True):
    nc = bacc.Bacc("TRN2", target_bir_lowering=False, debug=False)

    q_d = nc.dram_tensor("q", [HPC, S, D], FP32, kind="ExternalInput")
    k_d = nc.dram_tensor("k", [HPC, S, D], FP32, kind="ExternalInput")
    v_d = nc.dram_tensor("v", [HPC, S, D], FP32, kind="ExternalInput")
    ksc_d = (
        nc.dram_tensor("ksc", [D], FP32, kind="ExternalInput")
        if use_scale
        else None
    )
    mb_d = (
        nc.dram_tensor("mbias", [HPC, S], FP32, kind="ExternalInput")
        if use_mask
        else None
    )
    # direct [p, t*d] layout, bf16; host reassembles s = t*128 + p
    out_d = nc.dram_tensor("out", [HPC, P, NT * D], BF16, kind="ExternalOutput")

    AF = mybir.ActivationFunctionType
    ALU = mybir.AluOpType

    with tile.TileContext(nc) as tc:
        with (
            tc.tile_pool(name="constp", bufs=1) as constp,
            tc.tile_pool(name="dramp", bufs=4, space="DRAM") as dramp,
            tc.tile_pool(name="stagep", bufs=2) as stagep,
            tc.tile_pool(name="sqp", bufs=2) as sqp,
            tc.tile_pool(name="ssp", bufs=6) as ssp,
            tc.tile_pool(name="qnp", bufs=2) as qnp,
            tc.tile_pool(name="qtp", bufs=3) as qtp,
            tc.tile_pool(name="ktp", bufs=3) as ktp,
            tc.tile_pool(name="vbp", bufs=3) as vbp,
            tc.tile_pool(name="zmp", bufs=3) as zmp,
            tc.tile_pool(name="ptp", bufs=6) as ptp,
            tc.tile_pool(name="otsbp", bufs=3) as otsbp,
            tc.tile_pool(name="osbp", bufs=3) as osbp,
            tc.tile_pool(name="recp", bufs=4) as recp,
            tc.tile_pool(name="stp", bufs=2, space="PSUM") as stp,
            tc.tile_pool(name="otp", bufs=2, space="PSUM") as otp,
        ):
            # ---- constants ----
            tri = constp.tile([P, P], BF16, name="tri")
            nc.gpsimd.memset(tri[:], 1.0)
            # keep where col >= row (P^T layout: row=key j, col=query i)
            nc.gpsimd.affine_select(
                out=tri[:],
                in_=tri[:],
                pattern=[[1, P]],
                channel_multiplier=-1,
                base=0,
                compare_op=ALU.is_ge,
                fill=0.0,
            )
            identf = constp.tile([P, P], FP32, name="identf")
            make_identity(nc, identf[:])
            if DIAG_PE:
                # MASKVAL on the strict upper triangle (a < b): the mask
                # matmul writes its transpose = MASKVAL where i < j into
                # the diag S^T block before the QK matmul accumulates
                triM = constp.tile([P, P], BF16, name="triM")
                nc.gpsimd.memset(triM[:], MASKVAL)
                nc.gpsimd.affine_select(
                    out=triM[:],
                    in_=triM[:],
                    pattern=[[1, P]],
                    channel_multiplier=-1,
                    base=-1,
                    compare_op=ALU.is_ge,
                    fill=0.0,
                )
                identb = constp.tile([P, P], BF16, name="identb")
                make_identity(nc, identb[:])
            if use_scale:
                ksc_sb = constp.tile([P, 1], FP32, name="ksc_sb")
                for half in range(2):
                    nc.scalar.dma_start(
                        out=ksc_sb[half * D : (half + 1) * D, 0:1],
                        in_=ksc_d[:].rearrange("(d one) -> d one", one=1),
                    )

            pre = {}

            # ---- stage A1a: q/k loads into ONE combined staging tile.
            # contiguous-span layout: partition p holds rows 16p..16p+15
            # (4KB/partition descriptors at full DMA line rate). ----
            def loads_qk(h):
                st = {}
                xqk = stagep.tile([P, 2 * NT * D], FP32, tag="stage", name=f"xqk{h}")
                x4 = xqk.rearrange("p (w r d) -> p w r d", w=2, d=D)
                # head 0's q load rides the otherwise-idle scalar HWDGE ring
                qeng = nc.scalar if h == 0 else nc.sync
                qeng.dma_start(
                    out=x4[:, 0], in_=q_d[h].rearrange("(p r) d -> p r d", p=P)
                )
                nc.sync.dma_start(
                    out=x4[:, 1], in_=k_d[h].rearrange("(p r) d -> p r d", p=P)
                )
                st["xqk"] = xqk
                pre[h] = st
                if h == 0 and N_WARM > 0:
                    # PE warm-up: the HAM clock gate needs ~3.4us of
                    # sustained PE activity to lift the clock from 1.2 to
                    # 2.4 GHz, and re-throttles after ~3.4us idle.
                    warm = stp.tile([P, P], FP32, tag="st", name="warm")
                    junk = xqk[:, NT * D : NT * D + P // 2].bitcast(BF16)
                    for _ in range(N_WARM):
                        nc.tensor.matmul(
                            warm[:], tri[:], junk, start=True, stop=True,
                            skip_group_check=True,
                        )

            # ---- stage A1b: V loads via GpSimd SWDGE (cast fp32->bf16
            # straight into the (D+1)-strided layout; ones column memset). ----
            def loads_v(h):
                st = pre[h]
                vb = vbp.tile([P, NT * (D + 1)], BF16, tag="vb", name=f"vb{h}")
                vb3 = vb.rearrange("p (t c) -> p t c", c=D + 1)
                nc.gpsimd.dma_start(
                    out=vb3[:, :, 0:D],
                    in_=v_d[h].rearrange("(t p) d -> p t d", p=P),
                )
                nc.gpsimd.memset(vb3[:, :, D : D + 1], 1.0)
                if use_mask:
                    mbias = zmp.tile([P, NT], FP32, tag="zm", name=f"mb{h}")
                    nc.gpsimd.dma_start(
                        out=mbias[:], in_=mb_d[h].rearrange("(t p) -> p t", p=P)
                    )
                    st["mbias"] = mbias
                st["vb"] = vb

            # scratch round trip: write normalized bf16 rows with the d-span
            # DUPLICATED into a [S, 128] scratch, then one xbar transpose
            # back as [128, S] with both partition halves holding d (K=128
            # contraction keeps the PE activity monitor counting -> clock
            # stays 2.4 GHz).  dup=True takes a pre-duplicated [p,(r u d)]
            # SBUF tile (one contiguous 4KB/partition DMA - used on h0's
            # latency-critical path); dup=False writes the two 64-col
            # halves with two strided DMAs (baseline scheme; no extra DVE).
            def scratch_transpose(h, which, xnw, pool, dup):
                scratch = dramp.tile([S, P], BF16, tag="scratch", name=f"sc_{which}{h}")
                if dup:
                    nc.sync.dma_start(
                        out=scratch.rearrange("(p r) c -> p r c", p=P),
                        in_=xnw.rearrange("p (r c) -> p r c", c=2 * D),
                    )
                else:
                    for half in range(2):
                        nc.sync.dma_start(
                            out=scratch.rearrange("(p r) c -> p r c", p=P)[
                                :, :, half * D : (half + 1) * D
                            ],
                            in_=xnw.rearrange("p (r d) -> p r d", d=D),
                        )
                xt = pool.tile([P, S], BF16, tag=f"{which}T", name=f"{which}T{h}")
                nc.sync.dma_start_transpose(out=xt[:], in_=scratch[:])
                pre[h][f"{which}T"] = xt

            def newton_rsqrt(ss, width):
                # rsqrt(ss): Quake magic + 2 Newton iterations on DVE
                rs = ssp.tile([P, width], FP32, tag="ss", name="rs")
                rsi = rs.bitcast(mybir.dt.int32)
                nc.vector.tensor_scalar(
                    rsi, ss.bitcast(mybir.dt.int32), 1, None, ALU.arith_shift_right
                )
                nc.vector.tensor_scalar(
                    rsi, rsi, -1.0, float(0x5F3759DF), ALU.mult, ALU.add
                )
                tnw = ssp.tile([P, width], FP32, tag="ss", name="tnw")
                for _ in range(2):
                    nc.vector.tensor_mul(tnw[:], rs[:], rs[:])
                    nc.vector.tensor_mul(tnw[:], tnw[:], ss[:])
                    nc.vector.tensor_scalar(
                        tnw[:], tnw[:], -0.5, 1.5, ALU.mult, ALU.add
                    )
                    nc.vector.tensor_mul(rs[:], rs[:], tnw[:])
                return rs

            # ---- h0 preprocessing: split per tensor, k first, so kT's
            # scratch-write + transpose DMAs run under q's DVE chain.  The
            # combined variant serializes ~9us of DVE before any DMA. ----
            def normchain0():
                st = pre[0]
                xqk = st["xqk"]
                for which, off in (("k", NT), ("q", 0)):
                    sub = xqk[:, off * D : (off + NT) * D]
                    sqc = sqp.tile([P, NT * D], FP32, tag="sq", name=f"sqc0{which}")
                    nc.vector.tensor_mul(sqc[:], sub, sub)
                    ss = ssp.tile([P, NT], FP32, tag="ss", name=f"ss0{which}")
                    nc.vector.tensor_reduce(
                        out=ss[:],
                        in_=sqc.rearrange("p (g d) -> p g d", d=D),
                        axis=mybir.AxisListType.X,
                        op=ALU.add,
                    )
                    rs = newton_rsqrt(ss, NT)
                    # normalize into the u=0 slots of a dup layout, then a
                    # cheap bf16 copy fills u=1: enables a single
                    # contiguous scratch write on h0's critical path
                    xn = qnp.tile([P, NT * 2 * D], BF16, tag="qn", name=f"xn0{which}")
                    xn4 = xn.rearrange("p (g u d) -> p g u d", u=2, d=D)
                    rs_b = rs.rearrange("p (g one) -> p g one", one=1).broadcast_to(
                        [P, NT, D]
                    )
                    nc.vector.tensor_mul(
                        xn4[:, :, 0, :],
                        sub.rearrange("p (g d) -> p g d", d=D),
                        rs_b,
                    )
                    nc.vector.tensor_copy(xn4[:, :, 1, :], xn4[:, :, 0, :])
                    pool = qtp if which == "q" else ktp
                    scratch_transpose(0, which, xn[:], pool, dup=True)

            # ---- stage A2 (h>=1): combined q+k chain at LOWEST priority
            # so the greedy list-scheduler can't slot these 2.3us DVE ops
            # ahead of boundary-critical evac/diag work. ----
            def normchain(h):
                if h == 0:
                    return normchain0()
                with tc.high_priority(-10_000_000):
                    return normchain_body(h)

            def normchain_body(h):
                st = pre[h]
                xqk = st["xqk"]
                sqc = sqp.tile([P, 2 * NT * D], FP32, tag="sq", name=f"sqc{h}")
                nc.vector.tensor_mul(sqc[:], xqk[:], xqk[:])
                ss = ssp.tile([P, 2 * NT], FP32, tag="ss", name=f"ss{h}")
                nc.vector.tensor_reduce(
                    out=ss[:],
                    in_=sqc.rearrange("p (g d) -> p g d", d=D),
                    axis=mybir.AxisListType.X,
                    op=ALU.add,
                )
                rs = newton_rsqrt(ss, 2 * NT)
                xn = qnp.tile([P, 2 * NT * D], BF16, tag="qn", name=f"xn{h}")
                rs_b = rs.rearrange("p (g one) -> p g one", one=1).broadcast_to(
                    [P, 2 * NT, D]
                )
                nc.vector.tensor_mul(
                    xn.rearrange("p (g d) -> p g d", d=D),
                    xqk.rearrange("p (g d) -> p g d", d=D),
                    rs_b,
                )
                for which, off in (("q", 0), ("k", NT)):
                    xnw = xn[:, off * D : (off + NT) * D]
                    pool = qtp if which == "q" else ktp
                    scratch_transpose(h, which, xnw, pool, dup=False)

            # ---- stage B: the only op that waits on the transpose DMA.
            # Emitted after the previous half's loop so the wait is
            # near-zero by the time the DVE reaches it. ----
            def scale_kT(h):
                if not use_scale:
                    return
                kT = pre[h]["kT"]
                nc.vector.tensor_scalar(kT[:], kT[:], ksc_sb[:, 0:1], None, ALU.mult)

            # ---- attention ----
            def half_loop(h, ih, pend):
                st_h = pre[h]
                qT, kT, vb = st_h["qT"], st_h["kT"], st_h["vb"]
                mbias = st_h.get("mbias")
                ilo = ih * HALF
                ce = ilo + HALF
                njb = (ilo + HALF) // P  # 8 or 16
                oTh = otp.tile([D + 1, HALF], FP32, tag="ot", name=f"oT{h}_{ih}")
                live = {}
                oT_sb = otsbp.tile([D + 1, HALF], FP32, tag="otsb", name=f"osb{h}_{ih}")
                # ib slices are 128-col (512B) aligned: a transpose output
                # must not cross a PSUM bank boundary mid-instruction
                tp = otp.tile([P, 1024], FP32, tag="ot", name=f"tp{h}_{ih}")
                tp3 = tp.rearrange("p (ib c) -> p ib c", c=P)
                osb = osbp.tile([P, HALF // 2], BF16, tag="osb", name=f"osb2{h}_{ih}")
                osb3 = osb.rearrange("p (ib d) -> p ib d", d=D)

                def mk_tr(ib):
                    def f():
                        nc.tensor.transpose(
                            tp3[:, ib, 0 : D + 1],
                            oT_sb[:, ib * P : (ib + 1) * P],
                            identf[0 : D + 1, 0 : D + 1],
                        )
                    return f

                def mk_div():
                    def f():
                        rec = recp.tile([P, 8], FP32, tag="rec", name=f"rc{h}_{ih}")
                        rec1 = rec.rearrange("p (ib one) -> p ib one", one=1)
                        nc.vector.reciprocal(rec1, tp3[:, :, D : D + 1])
                        nc.vector.tensor_mul(
                            osb3[:], tp3[:, :, 0:D], rec1.broadcast_to([P, 8, D])
                        )
                    return f

                def mk_out():
                    def f():
                        nc.sync.dma_start(
                            out=out_d[h][:, ih * 8 * D : (ih + 1) * 8 * D],
                            in_=osb[:],
                        )
                    return f

                def bank_done(g):
                    nc.vector.tensor_copy(
                        oT_sb[:, 512 * g : 512 * g + 512],
                        oTh[:, 512 * g : 512 * g + 512],
                    )
                    for ib in range(4 * g, 4 * g + 4):
                        pend.append(mk_tr(ib))
                    if g == 1:
                        pend.append(mk_div())
                        pend.append(mk_out())

                last_jbs = {(ilo + 512 * g + 511) // P: g for g in range(2)}
                units = [[jb] for jb in range(njb)]
                if not use_mask:
                    units = [[jb] for jb in range(njb - 4)] + [
                        [njb - 4, njb - 3],
                        [njb - 2, njb - 1],
                    ]
                for step in range(len(units) + 1):
                    if step < len(units):
                        unit = units[step]
                        offs = []
                        cur = 0
                        for jb in unit:
                            cs = max(jb * P, ilo)
                            W = ce - cs
                            off = 0 if not offs else (cur if cur + W <= 512 else 512)
                            offs.append([jb, cs, W, off, None])
                            cur = off + W
                        stt = stp.tile(
                            [P, cur], FP32, tag="st", name=f"st{h}_{ih}_{step}"
                        )
                        for ent in offs:
                            jb, cs, W, off, _ = ent
                            isdiag = cs == jb * P
                            if DIAG_PE and isdiag:
                                # write MASKVAL into the upper triangle of
                                # the diag block; the first QK chunk then
                                # accumulates on top (start=False: masked
                                # cols add, virgin cols overwrite)
                                nc.tensor.matmul(
                                    stt[:, off : off + P],
                                    triM[:],
                                    identb[:],
                                    start=True,
                                    stop=False,
                                    skip_group_check=True,
                                )
                            n0 = cs
                            while n0 < ce:
                                w = min(1024 if SPAN2 else 512, ce - n0)
                                nc.tensor.matmul(
                                    stt[:, off + n0 - cs : off + n0 - cs + w],
                                    kT[:, jb * P : (jb + 1) * P],
                                    qT[:, n0 : n0 + w],
                                    start=not (DIAG_PE and isdiag and n0 == cs),
                                    stop=True,
                                    skip_group_check=DIAG_PE and isdiag,
                                )
                                n0 += w
                        pT = ptp.tile(
                            [P, cur], BF16, tag="pT", name=f"pT{h}_{ih}_{step}"
                        )
                        if OFF_TAILS and not use_mask and len(unit) > 1:
                            # tail units: exp on DVE via Schraudolph bit
                            # trick -- bf16 bits of e^(4x) ~ round(A*x+B)
                            pTi = pT.bitcast(mybir.dt.int16)
                            nc.vector.tensor_scalar(
                                pTi[:], stt[:], SCH_A, SCH_B, ALU.mult, ALU.add
                            )
                        else:
                            nc.scalar.activation(
                                pT[:],
                                stt[:],
                                AF.Exp,
                                scale=COSINE_SIM_SCALE / 2.0,
                                bias=mbias[:, unit[0] : unit[0] + 1]
                                if use_mask
                                else 0.0,
                            )
                        for ent in offs:
                            jb, cs, W, off, _ = ent
                            if cs == jb * P and not DIAG_PE:
                                nc.vector.tensor_mul(
                                    pT[:, off : off + P], pT[:, off : off + P], tri[:]
                                )
                            ent[4] = pT
                        live[step] = offs
                    if step >= 1:
                        for jb, cs, W, off, pT in live.pop(step - 1):
                            vslice = vb[:, jb * (D + 1) : (jb + 1) * (D + 1)]
                            n0 = cs
                            while n0 < ce:
                                if SPAN2:
                                    w = ce - n0
                                else:
                                    rel = n0 - ilo
                                    w = min(ilo + (rel // 512 + 1) * 512, ce) - n0
                                nc.tensor.matmul(
                                    oTh[:, n0 - ilo : n0 - ilo + w],
                                    vslice,
                                    pT[:, off + n0 - cs : off + n0 - cs + w],
                                    start=(jb == 0),
                                    stop=(jb == njb - 1),
                                    skip_group_check=True,
                                )
                                n0 += w
                            if jb in last_jbs:
                                bank_done(last_jbs[jb])
                    for _ in range(2):
                        if pend:
                            pend.popleft()()

            # ---- pipeline ----
            loads_qk(0)
            loads_v(0)
            normchain(0)
            scale_kT(0)
            pend = deque()
            for h in range(HPC):
                # h1's loads and chain are deferred (loads past
                # half_loop(0,0), chain past half_loop(0,1)): emitted any
                # earlier, their queue entries land in the middle of h0's
                # startup-critical chain and the load-wait semaphore
                # head-of-line-blocks the DVE FIFO for ~5us.
                if h + 1 < HPC:
                    loads_qk(h + 1)
                    loads_v(h + 1)
                half_loop(h, 0, pend)
                if h + 1 < HPC:
                    normchain(h + 1)
                half_loop(h, 1, pend)
                if h + 1 < HPC:
                    scale_kT(h + 1)
                del pre[h]
            while pend:
                pend.popleft()()

    nc.compile()
    return nc


def _variant_flags(q_scale, k_scale, mask):
    ksc = (np.asarray(q_scale, np.float32) * np.asarray(k_scale, np.float32))
    use_scale = not np.all(ksc == 1.0)
    use_mask = not np.all(np.asarray(mask))
    return use_scale, use_mask, ksc.astype(np.float32)


def make_in_maps(q, k, v, q_scale, k_scale, mask):
    qf = q.reshape(B * H, S, D)
    kf = k.reshape(B * H, S, D)
    vf = v.reshape(B * H, S, D)
    use_scale, use_mask, ksc = _variant_flags(q_scale, k_scale, mask)
    mbias_b = np.where(mask, 0.0, -1e30).astype(np.float32)  # [B, S]

    in_maps = []
    for c in range(N_CORES):
        heads = list(range(c * HPC, (c + 1) * HPC))
        m = {
            "q": np.ascontiguousarray(qf[heads]),
            "k": np.ascontiguousarray(kf[heads]),
            "v": np.ascontiguousarray(vf[heads]),
        }
        if use_scale:
            m["ksc"] = ksc
        if use_mask:
            m["mbias"] = np.ascontiguousarray(
                np.stack([mbias_b[bh // H] for bh in heads])
            )
        in_maps.append(m)
    return in_maps


_NC_CACHE = {}


def kernel(q, k, v, q_scale, k_scale, mask):
    q = np.asarray(q, dtype=np.float32)
    k = np.asarray(k, dtype=np.float32)
    v = np.asarray(v, dtype=np.float32)
    q_scale = np.asarray(q_scale, dtype=np.float32)
    k_scale = np.asarray(k_scale, dtype=np.float32)
    mask = np.asarray(mask)

    use_scale, use_mask, _ = _variant_flags(q_scale, k_scale, mask)
    key = (use_scale, use_mask)
    if key not in _NC_CACHE:
        _NC_CACHE[key] = build_nc(use_scale=use_scale, use_mask=use_mask)
    nc = _NC_CACHE[key]

    in_maps = make_in_maps(q, k, v, q_scale, k_scale, mask)
    res = run_bass_kernel_spmd(nc, in_maps, core_ids=list(range(N_CORES)))
    # out is [HPC, P, NT*D] bf16 with s = t*128 + p
    out = np.stack([np.asarray(r["out"]) for r in res.results])  # [8,4,P,NT*D]
    out = out.astype(np.float32).reshape(N_CORES * HPC, P, NT, D)
    out = out.transpose(0, 2, 1, 3).reshape(B, H, S, D)
    return np.ascontiguousarray(out)
